# revision 38
# baseline (speedup 1.0000x reference)
"""Trainium2 Bass kernel for causal multi-head attention (nn_Attention_5334349381821).

Problem: b=2, n=2048, dim=1024, 16 heads x 64 dim_head, RMSNorm + QKV + RoPE
(interleaved) + causal softmax attention + output projection.

Sharding: 8 cores = data-parallel on batch (2) x tensor-parallel on heads (4
groups of 4 heads). Each core computes a partial output [2048, 1024] through
its wo column-slice; host sums the 4 partials per batch element.

v2 design (chunk-streamed, bf16):
  - All inputs host-cast to bf16; norm_weight pre-folded into wq/wk/wv on host.
  - Work streams over 4 chunks of 512 tokens: QK-proj(c) + RoPE(c) + V(c),
    attention rows of chunk c (needs only k/v tiles <= c: causal), then output
    projection of chunk c.  Emission order per c:
      PROJ(c+1) before OUTPROJ(c) so the PE never waits on the normalize chain.
  - RMS scale s computed per-chunk (squares on DVE in bf16, sum-of-squares via
    ones-matmul, sqrt+recip) and folded into cos/sin tables (for q/k) and into
    the v copy (ACT copy with per-partition scale, via a DRAM-bounced sT_col).
  - Causal mask folded into the S accumulation group as an extra
    identity x (-30000 upper-triangle) matmul -> exp produces exact zeros; no
    vector-engine op in the S->exp->AV chain.
  - t-loop software pipelined: S(t+1) emitted before AV(t) so the ACT exp
    latency is hidden behind PE work.
  - Engine balance: exp + v-scale + qk copies on ACT/DVE, RoPE adds +
    broadcasts + half the normalize/staging on Pool (gpsimd).
All matmuls bf16 (fp32 PSUM accumulation).
"""

from contextlib import ExitStack

import numpy as np

import concourse.bass as bass
import concourse.tile as tile
from concourse import bacc, mybir
from concourse.bass_utils import run_bass_kernel_spmd

# Problem constants (hardcoded; kernel.py must be self-contained)
B = 2
N = 2048
DIM = 1024
HEADS = 16
DH = 64
N_CORES = 8
HEADS_PER_CORE = HEADS // (N_CORES // B)  # 4
M = HEADS_PER_CORE * DH  # 256 = per-core q/k/v width
RMS_EPS = 1.1920929e-07
ROPE_THETA = 10000.0

P = 128
F32 = mybir.dt.float32
FP16 = mybir.dt.float16

KT = DIM // P        # 8 k-tiles over dim
IT = N // P          # 16 token tiles of 128
NC = N // 512        # 4 chunks of 512 tokens
MT = M // P          # 2 m-tiles (= head-pairs)
CH = 512             # chunk width

SHUF_SWAP = [i ^ 1 for i in range(32)]
REPEATS = 1  # emit the body multiple times (for repeat-slope HW timing)
DEBUG_DUMPS = False

MULT = mybir.AluOpType.mult
ADD = mybir.AluOpType.add
EXPF = mybir.ActivationFunctionType.Exp
LNF = mybir.ActivationFunctionType.Ln


def build_program():
    nc = bacc.Bacc(
        "TRN2",
        target_bir_lowering=False,
        debug=False,
        enable_asserts=False,
        num_devices=N_CORES,
    )

    xT_d = nc.dram_tensor("xT", [DIM, N], FP16, kind="ExternalInput").ap()
    # wq|wk|wv packed on the m axis; cos|sin|cmask packed on the free axis —
    # one DMA each (HWDGE descriptor generation is ~0.65us per dma_start).
    wqkv_d = nc.dram_tensor("wqkv", [DIM, 3 * M], FP16, kind="ExternalInput").ap()
    woT_d = nc.dram_tensor("woT", [M, DIM], FP16, kind="ExternalInput").ap()
    ropec_d = nc.dram_tensor("ropec", [P, 2 * N + 256], FP16,
                             kind="ExternalInput").ap()
    out_d = nc.dram_tensor("out_part", [N, DIM], FP16, kind="ExternalOutput").ap()
    dbg = {}
    if DEBUG_DUMPS:
        for nm, shape, dt in [
            ("d_srow", [1, N], F32), ("d_stcol", [P, IT], F32),
            ("d_qT0", [P, N], FP16), ("d_kT0", [P, N], FP16),
            ("d_vaug", [P, IT * HEADS_PER_CORE * (DH + 1)], FP16),
            ("d_OT0", [P, N], FP16), ("d_cos", [P, N], FP16),
            ("d_wqkv", [P, KT * 3 * M], FP16), ("d_x0", [P, N], FP16),
            ("d_qraw", [P, CH], FP16), ("d_sbc", [P, CH], FP16),
        ]:
            dbg[nm] = nc.dram_tensor(nm, shape, dt, kind="ExternalOutput").ap()

    with tile.TileContext(nc) as tc:
        for _rep in range(REPEATS):
            _emit(nc, tc, xT_d, wqkv_d, woT_d, ropec_d, out_d, dbg)

    nc.compile()
    return nc


def _emit(nc, tc, xT_d, wqkv_d, woT_d, ropec_d, out_d, dbg={}):
    with ExitStack() as whole:
        persist = whole.enter_context(tc.tile_pool(name="persist", bufs=1))

        # ---------- persistent tiles ----------
        ropec = persist.tile([P, 2 * N + 256], FP16, name="ropec", tag="ropec")
        cos_t = ropec[:, 0:N]
        sin_t = ropec[:, N:2 * N]
        iden = ropec[:, 2 * N:2 * N + 128]
        maskadd = ropec[:, 2 * N + 128:2 * N + 256]
        ones_col = persist.tile([P, 1], FP16, name="ones_col", tag="ones_col")
        sT_col = persist.tile([P, IT], F32, name="sT_col", tag="sT_col")
        eps_t = persist.tile([1, 1], F32, name="eps_t", tag="eps_t")
        s_row = persist.tile([1, N], F32, name="s_row", tag="s_row")
        s_row_bf = persist.tile([1, N], FP16, name="s_row_bf", tag="s_row_bf")

        wqkv_sb = persist.tile([P, KT, 3 * M], FP16, name="wqkv_sb",
                               tag="wqkv_sb")
        wq_sb = wqkv_sb[:, :, 0:M]
        wk_sb = wqkv_sb[:, :, M:2 * M]
        wv_sb = wqkv_sb[:, :, 2 * M:3 * M]
        wo_sb = persist.tile([P, MT, DIM], FP16, name="wo_sb", tag="wo_sb")
        xT = [persist.tile([P, N], FP16, name=f"xT{kt}", tag=f"xT{kt}")
              for kt in range(KT)]

        qT = [persist.tile([P, N], FP16, name=f"qT{mt}", tag=f"qT{mt}")
              for mt in range(MT)]
        kTt = [persist.tile([P, N], FP16, name=f"kT{mt}", tag=f"kT{mt}")
               for mt in range(MT)]
        v_aug = persist.tile([P, IT, HEADS_PER_CORE, DH + 1], FP16,
                             name="v_aug", tag="v_aug")
        OT = [persist.tile([P, N], FP16, name=f"OT{mt}", tag=f"OT{mt}")
              for mt in range(MT)]

        # ---------- loads (ordered for time-to-first-matmul) ----------
        nc.vector.memset(ones_col[:], 1.0)
        nc.vector.memset(eps_t[:], RMS_EPS)
        # chunk-0 columns first (unblocks ph1(0)/proj(0)), weights
        # interleaved so wq is ready right when proj(0) starts, then the
        # rest of x
        for kt in range(4):
            nc.sync.dma_start(xT[kt][:, 0:CH], xT_d[kt * P:(kt + 1) * P, 0:CH])
        nc.sync.dma_start(wq_sb[:], wqkv_d[:, 0:M].rearrange("(o p) m -> p o m", p=P))
        for kt in range(4, KT):
            nc.sync.dma_start(xT[kt][:, 0:CH], xT_d[kt * P:(kt + 1) * P, 0:CH])
        nc.sync.dma_start(wk_sb[:], wqkv_d[:, M:2 * M].rearrange("(o p) m -> p o m", p=P))
        nc.sync.dma_start(wv_sb[:], wqkv_d[:, 2 * M:3 * M].rearrange("(o p) m -> p o m", p=P))
        nc.sync.dma_start(ropec[:], ropec_d[:])
        for kt in range(KT):
            nc.sync.dma_start(xT[kt][:, CH:N], xT_d[kt * P:(kt + 1) * P, CH:N])
        nc.sync.dma_start(wo_sb[:], woT_d.rearrange("(o p) d -> p o d", p=P))
        nc.gpsimd.memset(v_aug[:, :, :, DH:DH + 1], 1.0)

        # ---------- all pools in one scope ----------
        # PSUM banks: qkv 2 (qk/v accs + outproj) + s 4 (merged [P,2,CH]
        # bufs=2, also hosts the phase-1 ssq accumulators) + o 2 = 8.
        # GPSIMD cannot read PSUM: every PSUM consumer is DVE or ACT.
        with tc.tile_pool(name="ps_qkv", bufs=2, space="PSUM") as ps_qkv, \
             tc.tile_pool(name="ps_s", bufs=2, space="PSUM") as ps_s, \
             tc.tile_pool(name="ps_o", bufs=1, space="PSUM") as ps_o, \
             tc.tile_pool(name="ph1", bufs=3) as ph1, \
             tc.tile_pool(name="ph1b", bufs=2) as ph1b, \
             tc.tile_pool(name="rope", bufs=3) as rope, \
             tc.tile_pool(name="ptp", bufs=3) as ptp, \
             tc.tile_pool(name="dpool", bufs=2) as dpool, \
             tc.tile_pool(name="opool", bufs=2) as opool, \
             tc.tile_pool(name="dram_s", bufs=1, space="DRAM") as dram_pool:

            # ---------- phase 1: RMS scale per chunk, folded into cos/sin ----
            s_dram = dram_pool.tile([1, N], F32, name="s_dram", tag="s_dram")

            def ph1_chunk(c):
                cs = slice(c * CH, (c + 1) * CH)
                ssq_t = ps_o.tile([DH + 1, CH], F32, name=f"ssq{c}",
                                  tag=f"pO{c % 2}")
                ssq = ssq_t[0:1, :]
                for kt in range(KT):
                    sq = ph1.tile([P, CH], FP16, name="sq", tag="sq")
                    nc.vector.tensor_tensor(sq[:], xT[kt][:, cs], xT[kt][:, cs],
                                            MULT)
                    nc.tensor.matmul(ssq, ones_col[:], sq[:],
                                     start=(kt == 0), stop=(kt == KT - 1))
                # s = (ms+eps)^-1/2 as exp(-ln(ms+eps)/2): Ln/Exp/Copy all
                # live in one activation table (sqrt does not), so the ACT
                # engine never reloads tables mid-run.
                rt = ph1b.tile([1, CH], F32, name="rt", tag="rt")
                nc.scalar.activation(rt[:], ssq, LNF, bias=eps_t[:],
                                     scale=1.0 / DIM)
                nc.scalar.activation(s_row[:, cs], rt[:], EXPF, scale=-0.5)
                nc.vector.tensor_copy(s_row_bf[:, cs], s_row[:, cs])
                s_bc = ph1b.tile([P, CH], FP16, name="s_bc", tag="s_bc")
                nc.gpsimd.partition_broadcast(s_bc[:], s_row_bf[0:1, cs])
                if dbg and c == 0:
                    nc.sync.dma_start(dbg["d_sbc"], s_bc[:])
                nc.vector.tensor_tensor(cos_t[:, cs], cos_t[:, cs], s_bc[:],
                                        MULT)
                nc.vector.tensor_tensor(sin_t[:, cs], sin_t[:, cs], s_bc[:],
                                        MULT)
                # token-partition layout for the v scale: SBUF->SBUF
                # partition-crossing DMA corrupts on HW, so bounce via DRAM.
                # Must stay per-chunk: proj(c)'s v-scale reads sT_col[:, 4c:]
                # right after ph1(c), so the write has to be emitted here.
                nc.sync.dma_start(s_dram[:, cs], s_row[:, cs])
                nc.sync.dma_start(
                    sT_col[:, c * 4:(c + 1) * 4],
                    s_dram[0, cs].rearrange("(t p) -> p t", p=P),
                )

            def proj(c):
                cs = slice(c * CH, (c + 1) * CH)
                for w_sb, dst, mt in ((wq_sb, qT, 0), (wk_sb, kTt, 0),
                                      (wq_sb, qT, 1), (wk_sb, kTt, 1)):
                    acc = ps_qkv.tile([P, CH], F32, name="acc", tag="acc")
                    lhs = w_sb[:, :, mt * P:(mt + 1) * P]
                    for kt in range(KT):
                        nc.tensor.matmul(acc[:], lhs[:, kt, :], xT[kt][:, cs],
                                         start=(kt == 0), stop=(kt == KT - 1))
                    qraw = rope.tile([P, CH], FP16, name="qraw", tag="qraw")
                    nc.vector.tensor_copy(qraw[:], acc[:])
                    if dbg and c == 0 and mt == 0 and w_sb is wq_sb:
                        nc.sync.dma_start(dbg["d_qraw"], qraw[:])
                    sw = rope.tile([P, CH], FP16, name="sw", tag="sw")
                    nc.vector.stream_shuffle(sw[:], qraw[:], SHUF_SWAP)
                    t1 = rope.tile([P, CH], FP16, name="t1", tag="t1")
                    nc.vector.tensor_tensor(t1[:], qraw[:], cos_t[:, cs], MULT)
                    nc.vector.tensor_tensor(sw[:], sw[:], sin_t[:, cs], MULT)
                    nc.gpsimd.tensor_tensor(dst[mt][:, cs], t1[:], sw[:], ADD)
                for jt in range(4 * c, 4 * c + 4):
                    vp = ps_qkv.tile([P, CH], F32, name="vp", tag="acc")
                    for kt in range(KT):
                        nc.tensor.matmul(
                            vp[:, 0:M],
                            xT[kt][:, jt * P:(jt + 1) * P],
                            wv_sb[:, kt, :],
                            start=(kt == 0), stop=(kt == KT - 1),
                        )
                    nc.scalar.mul(
                        v_aug[:, jt, :, 0:DH],
                        vp[:, 0:M].rearrange("p (h e) -> p h e",
                                             h=HEADS_PER_CORE),
                        sT_col[:, jt:jt + 1],
                    )

            def attn(c):
                cs = slice(c * CH, (c + 1) * CH)
                tmax = 4 * (c + 1)
                for hp in range(MT):
                    pO = [
                        ps_o.tile([DH + 1, CH], F32, name=f"pO{h}", tag=f"pO{h}")
                        for h in range(2)
                    ]
                    pending = []  # software pipeline: AV(t) after S(t+1)

                    def flush():
                        for pt_t, t_, off_, nt_ in pending:
                            for h in range(2):
                                nc.tensor.matmul(
                                    pO[h][:, off_:off_ + nt_],
                                    v_aug[:, t_, hp * 2 + h, :],
                                    pt_t[:, h, 0:nt_],
                                    start=(t_ == 0), stop=(t_ == tmax - 1),
                                    skip_group_check=True,
                                )
                        pending.clear()

                    for t in range(tmax):
                        off = max(0, t - 4 * c) * P
                        nt = CH - off
                        i_lo = c * CH + off
                        diag = t >= 4 * c
                        ps_t = ps_s.tile([P, 2, CH], F32, name="sS", tag="sS")
                        for h in range(2):
                            base = h * DH
                            kslc = kTt[hp][base:base + DH, t * P:(t + 1) * P]
                            if not diag:
                                nc.tensor.matmul(
                                    ps_t[:, h, 0:nt], kslc,
                                    qT[hp][base:base + DH, i_lo:(c + 1) * CH],
                                    start=True, stop=True,
                                    tile_position=(base, 0),
                                )
                            else:
                                # diagonal block: S + causal mask-add share a
                                # group over cols 0:128; the rest is its own
                                nc.tensor.matmul(
                                    ps_t[:, h, 0:P], kslc,
                                    qT[hp][base:base + DH, i_lo:i_lo + P],
                                    start=True, stop=False,
                                    tile_position=(base, 0),
                                )
                                nc.tensor.matmul(
                                    ps_t[:, h, 0:P], iden, maskadd,
                                    start=False, stop=True,
                                )
                                if nt > P:
                                    nc.tensor.matmul(
                                        ps_t[:, h, P:nt], kslc,
                                        qT[hp][base:base + DH,
                                               i_lo + P:(c + 1) * CH],
                                        start=True, stop=True,
                                        tile_position=(base, 0),
                                    )
                        pt_t = ptp.tile([P, 2, CH], FP16, name="pt", tag="pt")
                        nc.scalar.activation(pt_t[:, :, 0:nt], ps_t[:, :, 0:nt],
                                             EXPF, scale=DH ** -0.5)
                        flush()
                        pending.append((pt_t, t, off, nt))
                    flush()
                    for h in range(2):
                        den = dpool.tile([1, CH], F32, name="den", tag="den")
                        nc.vector.reciprocal(den[:], pO[h][DH:DH + 1, :])
                        recb = dpool.tile([DH, CH], F32, name="recb",
                                          tag="recb")
                        nc.gpsimd.partition_broadcast(recb[:], den[:])
                        nc.vector.tensor_tensor(
                            OT[hp][h * DH:(h + 1) * DH, cs],
                            pO[h][0:DH, :], recb[:], MULT,
                        )

            def outproj(c):
                for pair in range(2):
                    it0 = 4 * c + 2 * pair
                    osb = opool.tile([P, 2, DIM], FP16, name="osb", tag="osb")
                    for u in range(2):
                        it = it0 + u
                        for dc in range(DIM // CH):
                            po = ps_qkv.tile([P, CH], F32, name="po", tag="acc")
                            for et in range(MT):
                                nc.tensor.matmul(
                                    po[:],
                                    OT[et][:, it * P:(it + 1) * P],
                                    wo_sb[:, et, dc * CH:(dc + 1) * CH],
                                    start=(et == 0), stop=(et == MT - 1),
                                )
                            dst = osb[:, u, dc * CH:(dc + 1) * CH]
                            if dc == 0:
                                nc.vector.tensor_copy(dst, po[:])
                            else:
                                nc.scalar.copy(dst, po[:])
                    nc.sync.dma_start(
                        out_d[it0 * P:(it0 + 2) * P, :].rearrange(
                            "(u p) d -> p u d", p=P),
                        osb[:],
                    )

            # ph1(c) interleaves just ahead of proj(c) so later chunks'
            # squares don't head-of-line-block the DVE queue; attn(0) runs
            # before proj(1) (whose x columns arrive late); proj runs a chunk
            # ahead of attn afterwards so the PE never waits on the RoPE
            # chain; outproj(c-1) between proj and attn keeps it off the tail.
            ph1_chunk(0)
            proj(0)
            ph1_chunk(1)
            attn(0)
            proj(1)
            for c in range(1, NC):
                if c + 1 < NC:
                    ph1_chunk(c + 1)
                    proj(c + 1)
                outproj(c - 1)
                attn(c)
            outproj(NC - 1)
            if dbg:
                nc.sync.dma_start(dbg["d_srow"], s_row[:])
                nc.sync.dma_start(dbg["d_stcol"], sT_col[:])
                nc.sync.dma_start(dbg["d_qT0"], qT[0][:])
                nc.sync.dma_start(dbg["d_kT0"], kTt[0][:])
                nc.sync.dma_start(
                    dbg["d_vaug"], v_aug.rearrange("p a b c -> p (a b c)"))
                nc.sync.dma_start(dbg["d_OT0"], OT[0][:])
                nc.sync.dma_start(dbg["d_cos"], cos_t)
                nc.sync.dma_start(
                    dbg["d_wqkv"], wqkv_sb.rearrange("p a b -> p (a b)"))
                nc.sync.dma_start(dbg["d_x0"], xT[0][:])


def _rope_tables():
    inv_freq = 1.0 / (ROPE_THETA ** (np.arange(0, DH, 2, dtype=np.float64) / DH))
    t = np.arange(N, dtype=np.float64)
    freqs = t[:, None] * inv_freq[None, :]  # [N, 32]
    cos = np.cos(freqs)
    sin = np.sin(freqs)
    rows = np.arange(P)
    tidx = (rows % DH) // 2
    cos_t = cos[:, tidx].T  # [128, N]
    sign = np.where(rows % 2 == 0, -1.0, 1.0)
    sin_t = (sin[:, tidx] * sign[None, :]).T
    return cos_t, sin_t


def shard_inputs(tokens, norm_weight, wq, wk, wv, wo):
    """Build the 8 per-core input dicts (pure numpy layout prep, fp16)."""
    BF = np.float16
    tokens = np.asarray(tokens, dtype=np.float32)
    norm_weight = np.asarray(norm_weight, dtype=np.float64)
    wq, wk, wv, wo = (np.asarray(w, dtype=np.float64) for w in (wq, wk, wv, wo))

    cos_t, sin_t = _rope_tables()
    ii = np.arange(P)
    mask = np.where(ii[None, :] >= ii[:, None], 0.0, -30000.0)  # [j, i_local]
    ropec = np.ascontiguousarray(
        np.concatenate([cos_t, sin_t, np.eye(P), mask], axis=1).astype(BF)
    )

    in_maps = []
    for c in range(N_CORES):
        b = c // (N_CORES // B)
        g = c % (N_CORES // B)
        sl = slice(g * M, (g + 1) * M)
        wq_eff = (wq[sl, :] * norm_weight[None, :]).T  # [dim, M]
        wk_eff = (wk[sl, :] * norm_weight[None, :]).T
        wv_eff = (wv[sl, :] * norm_weight[None, :]).T
        in_maps.append({
            "xT": np.ascontiguousarray(tokens[b].T.astype(BF)),
            "wqkv": np.ascontiguousarray(
                np.concatenate([wq_eff, wk_eff, wv_eff], axis=1).astype(BF)
            ),
            "woT": np.ascontiguousarray(wo[:, sl].T.astype(BF)),
            "ropec": ropec,
        })
    return in_maps


_PROGRAM = None


def _get_program():
    global _PROGRAM
    if _PROGRAM is None:
        _PROGRAM = build_program()
    return _PROGRAM


def run(tokens, norm_weight, wq, wk, wv, wo, trace=False, **run_kwargs):
    nc = _get_program()
    in_maps = shard_inputs(tokens, norm_weight, wq, wk, wv, wo)
    res = run_bass_kernel_spmd(
        nc, in_maps, core_ids=list(range(N_CORES)), trace=trace, **run_kwargs
    )
    parts = [r["out_part"] for r in res.results]
    out = np.zeros((B, N, DIM), dtype=np.float64)
    for c in range(N_CORES):
        out[c // (N_CORES // B)] += parts[c].astype(np.float64)
    return out.astype(np.float32), res


def kernel(tokens, norm_weight, wq, wk, wv, wo):
    out, _ = run(tokens, norm_weight, wq, wk, wv, wo)
    return out
